# revision 2
# baseline (speedup 1.0000x reference)
"""Contrastive-loss kernel for Trainium2 (Bass/Tile), 8-core SPMD, v2.

Reference semantics (B=4, N=4096, D=128, T=0.1):
    emb_n = emb / max(||emb||, 1e-12)
    pos_sim[b,n] = dot(emb_n[b,n], emb_n[b, pos_idx[b,n]]) / T
    loss = mean(softplus(-pos_sim)) + mean(softplus(neg_sim))

Only two gathered dot products per row are needed.  Each of the 8 cores
handles half the rows of one batch element (core = 2*b + h).

v2 design (vs the all-DVE tree-reduction v1):
  * dma_gather(transpose=True) lands partner rows in [d=128 partitions,
    row=free] layout, so per-row reductions over D become partition-dim
    reductions on the otherwise-idle PE.
  * Chunk c's matmul uses a one-hot-column stationary (ones in column c)
    and accumulates into a [16, 128] PSUM region, so row sums land
    [chunk-on-partition, row-in-chunk] -- directly chain-consumable, no
    PSUM reshape.  PE p-state is kept warm with filler matmuls so the
    real ones run at full clock.
  * own rows ship transposed as fp8_e4m3 (half DMA), upconverted on ACT;
    gathers stay bf16 (gather elem must be a multiple of 256B).
  * Scalar chain per direction, [16,128] f32 on PSUM/SBUF:
    ln(ssq_g*ssq_own), exp(-x/2), *dot, exp(+-x/T), ln(1+x)+accum.
    The sign of z is folded into the Exp scale per direction.
  * Gathers split pos 1024+1024, neg 1024+768+256: the small last chunk
    keeps the critical tail (last gather -> product -> matmul -> chain)
    short while Pool desc-gen still stays ahead of the DMA stream.
  * Output: [32,1] f32 partial sums per core ([0:16] pos chunk sums,
    [16:32] neg); host sums all / (B*N).
"""

import numpy as np

B, N, D = 4, 4096, 128
NCORES = 8
HALF = N // 2          # rows per core
NCH = HALF // 128      # 16 chunks of 128 rows
NREG = 64              # psum region partition count (chunks per stream)
RW = HALF // NREG      # psum region width (rows per chunk) = 32
TEMP = 0.1
# (direction, start, count) gather chunks; count % 128 == 0
GCH = (("pos", 0, 1024), ("pos", 1024, 1024),
       ("neg", 0, 1024), ("neg", 1024, 768), ("neg", 1792, 256))

_PROG = None


def _pin_act_table(table_name="natural_log_exp_and_others"):
    """Make Square/Ln/Exp resolve only to `table_name` so the act-table-load
    pass emits a single table load instead of ping-ponging between tables."""
    import functools
    import concourse.hw_specs as hw_specs
    import concourse.bacc as bacc
    import concourse.mybir as mybir

    if getattr(_pin_act_table, "_done", False):
        return
    orig = hw_specs.get_activation_tables
    AF = mybir.ActivationFunctionType
    pinned = {AF.Square, AF.Ln, AF.Exp}

    @functools.cache
    def patched(arch):
        return {k: (v if k == table_name else v - pinned)
                for k, v in orig(arch).items()}

    hw_specs.get_activation_tables = patched
    bacc.get_activation_tables = patched
    _pin_act_table._done = True


def _build_program():
    import concourse.bacc as bacc
    import concourse.tile as tile
    import concourse.mybir as mybir

    _pin_act_table()

    f32 = mybir.dt.float32
    bf16 = mybir.dt.bfloat16
    fp8 = mybir.dt.float8e4
    i16 = mybir.dt.int16
    mult = mybir.AluOpType.mult
    AF = mybir.ActivationFunctionType

    nc = bacc.Bacc("TRN2", target_bir_lowering=False)

    table = nc.dram_tensor("table", [N, D], bf16, kind="ExternalInput")
    own = nc.dram_tensor("own", [128, HALF], bf16, kind="ExternalInput")
    posi = nc.dram_tensor("pos_idx", [128, 128], i16, kind="ExternalInput")
    negi = nc.dram_tensor("neg_idx", [128, 128], i16, kind="ExternalInput")
    out = nc.dram_tensor("partial", [64, 2], f32, kind="ExternalOutput")

    with tile.TileContext(nc) as tc:
        with tc.tile_pool(name="p", bufs=1) as pool, \
             tc.psum_pool(name="ps", bufs=1) as ppool:
            # --- input DMAs (SP HWDGE): idx first so desc-gen starts early
            idx_t = {}
            for name, src in (("pos", posi), ("neg", negi)):
                t = pool.tile([128, 128], i16, tag=f"idx{name}")
                nc.sync.dma_start(out=t[:], in_=src[:])
                idx_t[name] = t
            ownb = pool.tile([128, HALF], bf16)
            nc.sync.dma_start(out=ownb[:], in_=own[:])

            # --- constants (DVE memsets): sliding-window one-hot
            # stationaries: sta(c) = Z[:, NREG-c : 2*NREG-c] has ones in
            # column c only (Z is zero except column NREG).
            Z = pool.tile([128, 3 * NREG], bf16)
            nc.vector.memset(Z[:], 0.0)
            nc.vector.memset(Z[:, NREG:NREG + 1], 1.0)
            warm = pool.tile([128, 512], bf16)
            nc.vector.memset(warm[:], 0.0)

            # --- PSUM regions: one tile per region, so chain reads never
            # create false tile-level WAR/RAW deps against later matmuls
            reg = {}
            for rname in ("sqpos", "sqneg", "sqown", "prpos", "prneg"):
                rt = ppool.tile([NREG, RW], f32, tag=f"P{rname}")
                reg[rname] = rt[:]
            Pwarm = ppool.tile([1, 512], f32)

            def warmup(k):
                for _ in range(k):
                    nc.tensor.matmul(out=Pwarm[:],
                                     lhsT=Z[:, NREG:NREG + 1], rhs=warm[:])

            def reduce_mm(region, src, c, start, stop):
                # stationary one-hot in column c selects psum partition c
                nc.tensor.matmul(out=region,
                                 lhsT=Z[:, NREG - c:2 * NREG - c],
                                 rhs=src[:, RW * c:RW * (c + 1)],
                                 start=start, stop=stop, skip_group_check=True)

            # build PE ramp before the first real matmuls
            warmup(10)

            # --- gathers (Pool SWDGE), transpose=True -> [d, row] layout
            gpos = pool.tile([128, HALF], bf16)
            gneg = pool.tile([128, HALF], bf16)
            gt = {"pos": gpos, "neg": gneg}
            for name, start, cnt in GCH:
                nc.gpsimd.dma_gather(
                    out_ap=gt[name][:, start:start + cnt].rearrange(
                        "p (a n) -> p a n", a=1),
                    in_ap=table[:],
                    idxs_ap=idx_t[name][:, start // 16:(start + cnt) // 16],
                    num_idxs=cnt,
                    num_idxs_reg=cnt,
                    elem_size=D,
                    transpose=True,
                    single_packet=False,
                )

            # --- own: square on DVE, PE -> ssq_own
            # (tile_wait_until hints give the scheduler the real data-arrival
            # timeline; without them it interleaves engine queues so that
            # monotonic completion sems make early work wait on late work)
            sqo = pool.tile([128, HALF], bf16)
            with tc.tile_wait_until(0.0049):
                for k in range(2):
                    sl = slice(k * 1024, (k + 1) * 1024)
                    nc.vector.tensor_tensor(out=sqo[:, sl], in0=ownb[:, sl],
                                            in1=ownb[:, sl], op=mult)
            with tc.tile_wait_until(0.0061):
                for c in range(NREG):
                    reduce_mm(reg["sqown"], sqo, c, start=(c == 0),
                              stop=(c == NREG - 1))
                warmup(2)
            sqow_sb = pool.tile([NREG, RW], f32)
            with tc.tile_wait_until(0.007):
                nc.vector.tensor_copy(sqow_sb[:], reg["sqown"])

            # --- scalar chains ([NREG, RW] f32; pos hidden, neg on tail)
            acc2 = pool.tile([NREG, 2], f32)

            def chain(i, name, hints):
                # per-op schedule hints (us)
                def at(k):
                    return tc.tile_wait_until(hints[k] / 1000.0)
                sprod = pool.tile([NREG, RW], f32, tag=f"spr{name}")
                with at(0):
                    nc.vector.tensor_tensor(out=sprod[:],
                                            in0=reg["sq" + name],
                                            in1=sqow_sb[:], op=mult)
                lnp = pool.tile([NREG, RW], f32, tag=f"lnp{name}")
                with at(1):
                    nc.scalar.activation(lnp[:], sprod[:], AF.Ln)
                rsq = pool.tile([NREG, RW], f32, tag=f"rsq{name}")
                with at(2):
                    nc.scalar.activation(rsq[:], lnp[:], AF.Exp, scale=-0.5)
                cz = pool.tile([NREG, RW], f32, tag=f"cz{name}")
                with at(3):
                    nc.vector.tensor_tensor(out=cz[:], in0=rsq[:],
                                            in1=reg["pr" + name], op=mult)
                ez = pool.tile([NREG, RW], f32, tag=f"ez{name}")
                sign = -1.0 if name == "pos" else 1.0
                with at(4):
                    nc.scalar.activation(ez[:], cz[:], AF.Exp,
                                         scale=sign / TEMP)
                sl = pool.tile([NREG, RW], f32, tag=f"sl{name}")
                with at(5):
                    # softplus(z) = ln(exp(z)+1): +1 folds into Ln's bias
                    nc.scalar.activation(sl[:], ez[:], AF.Ln, bias=1.0,
                                         accum_out=acc2[:, i:i + 1])

            # --- per gather chunk: product, square, PE reductions
            prt, sqt = {}, {}
            for name in ("pos", "neg"):
                prx = pool.tile([128, HALF], bf16, tag=f"pr{name}")
                sqx = pool.tile([128, HALF], bf16, tag=f"sq{name}")
                prt[name], sqt[name] = prx, sqx
            # (sq-ready, prod-ready, sq-mm, pr-mm) estimates per chunk, us.
            # The last chunks order squares before products so the ssq
            # regions (which gate the chain head) close first.
            GT = ((7.45, 7.45, 8.6, 8.6), (8.9, 8.9, 9.95, 9.95),
                  (10.35, 10.35, 11.3, 11.5), (11.45, 12.2, 12.3, 12.75),
                  (11.8, 12.45, 12.45, 12.9))
            for gi, (name, start, cnt) in enumerate(GCH):
                g, pr, sq = gt[name], prt[name], sqt[name]
                sl = slice(start, start + cnt)
                t_sq, t_pr, t_smm, t_pmm = GT[gi]
                with tc.tile_wait_until(t_sq / 1000.0):
                    if gi != 4:
                        nc.scalar.activation(sq[:, sl], g[:, sl], AF.Square)
                    else:   # last chunk: DVE; keeps the tail short
                        nc.vector.tensor_tensor(out=sq[:, sl], in0=g[:, sl],
                                                in1=g[:, sl], op=mult)
                with tc.tile_wait_until(t_pr / 1000.0):
                    nc.vector.tensor_tensor(out=pr[:, sl], in0=ownb[:, sl],
                                            in1=g[:, sl], op=mult)
                with tc.tile_wait_until(t_smm / 1000.0):
                    for c in range(start // RW, (start + cnt) // RW):
                        reduce_mm(reg["sq" + name], sq, c,
                                  start=c == 0, stop=c == NREG - 1)
                with tc.tile_wait_until(t_pmm / 1000.0):
                    for c in range(start // RW, (start + cnt) // RW):
                        reduce_mm(reg["pr" + name], pr, c,
                                  start=c == 0, stop=c == NREG - 1)
                    if gi < 2:
                        warmup(2)
                if gi == 1:
                    chain(0, "pos", (10.1, 10.15, 11.5, 11.75, 12.3, 12.55))
                if gi == 2:
                    with tc.tile_wait_until(0.0096):
                        warmup(4)

            chain(1, "neg", (12.95, 13.1, 13.45, 13.8, 14.05, 14.45))
            with tc.tile_wait_until(0.0145):
                nc.sync.dma_start(out=out[:], in_=acc2[:])

    nc.compile()
    return nc


def _get_program():
    global _PROG
    if _PROG is None:
        _PROG = _build_program()
    return _PROG


def _wrap_idx(rows):
    """Index layout for dma_gather: gather slot i (= output column i in
    transpose mode) reads table row unwrapped[i]; the Q7 cores read
    idxs[p, s] = unwrapped[s*16 + p] from 16 partitions, replicated x8."""
    unwrapped = np.asarray(rows, dtype=np.int16)            # [2048]
    wrapped = unwrapped.reshape(-1, 16).T                   # [16, 128]
    return np.tile(wrapped, (8, 1)).astype(np.int16)        # [128, 128]


def _shard_inputs(embeddings, positive_pairs, negative_pairs):
    import ml_dtypes

    emb = np.asarray(embeddings, dtype=np.float32)
    emb_bf = emb.astype(ml_dtypes.bfloat16)
    pos = np.asarray(positive_pairs).reshape(B, N)
    neg = np.asarray(negative_pairs).reshape(B, N)

    in_maps = []
    for c in range(NCORES):
        b, h = divmod(c, 2)
        own = emb[b, h * HALF:(h + 1) * HALF]               # [HALF, D] f32
        in_maps.append({
            "table": np.ascontiguousarray(emb_bf[b]),
            "own": np.ascontiguousarray(
                own.T.astype(ml_dtypes.bfloat16)),          # [128, HALF]
            "pos_idx": _wrap_idx(pos[b, h * HALF:(h + 1) * HALF]),
            "neg_idx": _wrap_idx(neg[b, h * HALF:(h + 1) * HALF]),
        })
    return in_maps


def kernel(embeddings, positive_pairs, negative_pairs):
    from concourse.bass_utils import run_bass_kernel_spmd

    nc = _get_program()
    in_maps = _shard_inputs(embeddings, positive_pairs, negative_pairs)
    res = run_bass_kernel_spmd(nc, in_maps, core_ids=list(range(NCORES)))
    total = sum(r["partial"].astype(np.float64).sum() for r in res.results)
    return np.float32(total / (B * N))
